# revision 4
# baseline (speedup 1.0000x reference)
"""Trainium2 Bass kernel for nn_BatchMinigrid: batched FPV render.

v2 strategy (per core, 4096 envs), pure data parallel across 8 cores:

- Host packs each rotated 7x7 window cell into ONE byte
  (v = ch0 | ch1<<2 | ch2<<4 | closed<<6, wall pad byte = 106) and uploads
  it CELL-MAJOR: cm[cell 0:49, env 0:4096] u8. All value math (closed
  thresholds, the 5-step tanh visibility fixed point in f32, the final
  conv + masking and the masked output bytes) runs on device; the host
  contributes layout/addressing only (crop + rot90 byte moves), extending
  the v1 envelope (v1 already packed bytes + computed DMA indices on host).
  This removes the 32-call SWDGE indirect gather spine (~35.5us of Pool
  time at 994ns fixed cost per 128-descriptor call -- measured; multi-
  column offset APs return garbage on HW, and InstDMAGatherAnt cannot be
  encoded by walrus outside Bacc), and removes all 80 PE transposes.

- Dual-band cell-major layout, KR=98 rows: pair (A=even, B=odd unit):
    stA: rows 0:49  = closed_A (f32), rows 49:98 = t_A
    stB: rows 0:49  = t_B,            rows 49:98 = closed_B
  One PSUM z tile + one ACT tanh [0:98] serves BOTH units per step.
  lhs_it [98,98]: cols 0:49 = [-0.01W; W] (A), cols 49:98 = [W; -0.01W].

- SBUF cm tile holds TWO band copies; the second is written with a 512-col
  shift so ONE tensor_scalar over rows 0:98 at col X=512(2q+1) yields the
  pair's open mask (rows 0:49 = open_B at X, rows 49:98 = open_A at X-512).

- Final: per-band 49-row matmuls (bf16, sign-exact) into PSUM, then one
  fused scalar_tensor_tensor per unit: out_u8 = (zf > 0) * cm_byte.
  Output [49, 4096] u8 cell-major; host transposes + unpacks channels
  ((m*v)>>2k & 3 == m*ch_k exactly since the mask is 0/1).
"""
import os
import numpy as np
import ml_dtypes
from contextlib import ExitStack

import concourse.bass as bass
import concourse.tile as tile
from concourse import mybir
from concourse.bass_utils import run_bass_kernel_spmd

P = 128
NENV = 4096          # envs per core
EB = 512             # envs per matmul column block (unit)
NU = 8               # units per core
KB = 49              # band rows (window cells)
TB = 64              # upper band base (PE tile bases must be 0/32/64)
KR = 113             # contraction rows (49 + 15 zero + 49)

LAST_RESULTS = {}    # test harness introspection


# ----------------------------------------------------------------- waitsplit
def _split_excess_waits(nc, limit=1):
    n_split = 0
    for fn in nc.m.functions:
        for blk in fn.blocks:
            insts = blk.instructions
            i = 0
            while i < len(insts):
                inst = insts[i]
                si = getattr(inst, "sync_info", None)
                if si is not None and si.on_wait and len(si.on_wait) > limit:
                    waits = list(si.on_wait)
                    si.on_wait.clear()
                    si.on_wait.extend(waits[-limit:])
                    rest = waits[:-limit]
                    pos = i
                    for j in range(0, len(rest), limit):
                        nop = mybir.InstNoOp(
                            name=f"{inst.name}_wsplit{j}",
                            engine=inst.engine,
                            bass_nofuse=True,
                            sync_info=mybir.SyncInfo(
                                on_wait=rest[j:j + limit], on_update=[]),
                        )
                        insts.insert(pos, nop)
                        pos += 1
                        i += 1
                        n_split += 1
                i += 1
    return n_split


# ----------------------------------------------------------------- builder
def build_nc():
    f32 = mybir.dt.float32
    bf16 = mybir.dt.bfloat16
    u8 = mybir.dt.uint8
    nc = bass.Bass()

    cm = nc.dram_tensor("cm", [KB, NENV], u8, kind="ExternalInput")
    lhs_it = nc.dram_tensor("lhs_it", [KR, 2 * KB], mybir.dt.float32,
                            kind="ExternalInput")
    lhs_fin = nc.dram_tensor("lhs_fin", [KR, KB], bf16, kind="ExternalInput")
    w27 = nc.dram_tensor("w27", [KR, 1], mybir.dt.float32,
                         kind="ExternalInput")
    out = nc.dram_tensor("out", [KB, NENV], u8, kind="ExternalOutput")

    with tile.TileContext(nc) as tc, ExitStack() as ctx:
        const = ctx.enter_context(tc.tile_pool(name="const", bufs=1))
        cmp_ = ctx.enter_context(tc.tile_pool(name="cmp", bufs=1))
        stp = ctx.enter_context(tc.tile_pool(name="stp", bufs=1))
        opsp = ctx.enter_context(tc.tile_pool(name="opsp", bufs=1))
        thp = ctx.enter_context(tc.tile_pool(name="thp", bufs=3))
        outp = ctx.enter_context(tc.tile_pool(name="outp", bufs=1))
        psZ = ctx.enter_context(tc.tile_pool(name="psZ", bufs=4, space="PSUM"))
        psF = ctx.enter_context(tc.tile_pool(name="psF", bufs=4, space="PSUM"))

        TS = nc.vector.tensor_scalar
        GTS = nc.gpsimd.tensor_scalar
        TT = nc.vector.tensor_tensor
        GTT = nc.gpsimd.tensor_tensor
        STT = nc.vector.scalar_tensor_tensor
        Alu = mybir.AluOpType
        ACTF = mybir.ActivationFunctionType

        # ---------------- inputs
        lhs_it_t = const.tile([P, 2 * KB], f32)
        nc.sync.dma_start(out=lhs_it_t[0:KR, :], in_=lhs_it[:])
        lhs_fin_t = const.tile([P, KB], bf16)
        nc.sync.dma_start(out=lhs_fin_t[0:KR, :], in_=lhs_fin[:])
        w27_t = const.tile([P, 1], f32)
        nc.sync.dma_start(out=w27_t[0:KR, :], in_=w27[:])

        # cm tile: band1 rows 0:49 = cm[:, c]; band2 rows 49:98 shifted by
        # +512 cols (cmt[49:98, c] = cm[:, c-512]) so pair-level 98-row APs
        # line up (see module docstring).
        cmt = cmp_.tile([P, NENV + EB], u8)
        nc.sync.dma_start(out=cmt[0:KB, 0:NENV], in_=cm[:])
        nc.sync.dma_start(out=cmt[TB:KR, EB:NENV + EB], in_=cm[:])

        sts = [None] * NU
        opss = [None] * 4
        mbs = [None] * NU
        outbuf = outp.tile([P, NENV], u8)

        # ---------------- front end per pair (quad q = units 2q, 2q+1)
        def front(q):
            ua, ub = 2 * q, 2 * q + 1
            xa, xb = ua * EB, ub * EB
            stA = stp.tile([P, EB], f32, tag=f"st{ua}", name=f"st{ua}")
            stB = stp.tile([P, EB], f32, tag=f"st{ub}", name=f"st{ub}")
            ops = opsp.tile([P, EB], bf16, tag=f"ops{q}", name=f"ops{q}")
            sts[ua], sts[ub] = stA, stB
            opss[q] = ops
            with nc.named_scope("front"):
                # zero gap band first (rows 32:49 rewritten by closed below;
                # engine partition bases must be quarter-aligned)
                nc.scalar.memzero(stA[32:TB, :])
                nc.scalar.memzero(stB[32:TB, :])
                # closed bands straight to f32 st
                GTS(out=stA[0:KB, :], in0=cmt[0:KB, xa:xa + EB],
                    scalar1=64.0, scalar2=None, op0=Alu.is_ge)
                GTS(out=stB[TB:KR, :], in0=cmt[TB:KR, xb + EB:xb + 2 * EB],
                    scalar1=64.0, scalar2=None, op0=Alu.is_ge)
                # pair open mask in one op (band trick)
                TS(out=ops[0:KR, :], in0=cmt[0:KR, xb:xb + EB],
                   scalar1=64.0, scalar2=None, op0=Alu.is_lt)
                # t-band init: tanh(W[:,27]) * open
                nc.scalar.activation(
                    out=stA[TB:KR, :], in_=ops[TB:KR, :],
                    func=ACTF.Copy, scale=w27_t[TB:KR, :])
                nc.scalar.activation(
                    out=stB[0:KB, :], in_=ops[0:KB, :],
                    func=ACTF.Copy, scale=w27_t[0:KB, :])

        # ---------------- iterations
        def iter_l(l, q):
            ua, ub = 2 * q, 2 * q + 1
            fn = ACTF.Tanh if l < 5 else ACTF.Relu
            with nc.named_scope(f"iter{l}"):
                zq = psZ.tile([P, EB], mybir.dt.float32, tag="z",
                              name=f"z{q}_{l}")
                th = thp.tile([P, EB], mybir.dt.float32, tag="th",
                              name=f"th{q}_{l}")
                if l < 5:
                    outa, outb = sts[ua], sts[ub]
                else:
                    outa = stp.tile([P, EB], bf16, tag=f"mb{ua}",
                                    name=f"mb{ua}")
                    outb = stp.tile([P, EB], bf16, tag=f"mb{ub}",
                                    name=f"mb{ub}")
                    mbs[ua], mbs[ub] = outa, outb
                nc.tensor.matmul(
                    out=zq[TB:KR, :], lhsT=lhs_it_t[0:KR, 0:KB],
                    rhs=sts[ua][0:KR, :], start=True, stop=True)
                nc.tensor.matmul(
                    out=zq[0:KB, :], lhsT=lhs_it_t[0:KR, KB:2 * KB],
                    rhs=sts[ub][0:KR, :], start=True, stop=True)
                nc.scalar.activation(out=th[0:KR, :], in_=zq[0:KR, :],
                                     func=fn)
                TT(out=outa[TB:KR, :], in0=th[TB:KR, :],
                   in1=opss[q][TB:KR, :], op=Alu.mult)
                GTT(out=outb[0:KB, :], in0=th[0:KB, :],
                    in1=opss[q][0:KB, :], op=Alu.mult)

        # ---------------- final conv + fused mask-multiply
        def final(q):
            ua, ub = 2 * q, 2 * q + 1
            xa, xb = ua * EB, ub * EB
            with nc.named_scope("final"):
                zfA = psF.tile([P, EB], mybir.dt.float32, tag="zf",
                               name=f"zfA{q}")
                zfB = psF.tile([P, EB], mybir.dt.float32, tag="zf",
                               name=f"zfB{q}")
                nc.tensor.matmul(
                    out=zfA[0:KB, :], lhsT=lhs_fin_t[TB:KR, :],
                    rhs=mbs[ua][TB:KR, :], start=True, stop=True)
                nc.tensor.matmul(
                    out=zfB[0:KB, :], lhsT=lhs_fin_t[0:KB, :],
                    rhs=mbs[ub][0:KB, :], start=True, stop=True)
                STT(out=outbuf[0:KB, xa:xa + EB], in0=zfA[0:KB, :],
                    scalar=0.0, in1=cmt[0:KB, xa:xa + EB],
                    op0=Alu.is_gt, op1=Alu.mult)
                STT(out=outbuf[0:KB, xb:xb + EB], in0=zfB[0:KB, :],
                    scalar=0.0, in1=cmt[0:KB, xb:xb + EB],
                    op0=Alu.is_gt, op1=Alu.mult)
                nc.sync.dma_start(out=out[:, xa:xa + 2 * EB],
                                  in_=outbuf[0:KB, xa:xa + 2 * EB])

        for q in range(4):
            front(q)
        # wavefront: quad q's layer l at wave q + l, finals at wave q + 6
        for w in range(2, 10):
            for q in range(4):
                l = w - q
                if 2 <= l <= 5:
                    iter_l(l, q)
                elif l == 6:
                    final(q)

    _split_excess_waits(nc)
    return nc


# ----------------------------------------------------------------- host side
def _conv_matrix(w):
    w = np.asarray(w, np.float32).reshape(3, 3)
    W = np.zeros((49, 49), np.float32)
    for i in range(7):
        for j in range(7):
            for di in (-1, 0, 1):
                for dj in (-1, 0, 1):
                    ii, jj = i + di, j + dj
                    if 0 <= ii < 7 and 0 <= jj < 7:
                        W[i * 7 + j, ii * 7 + jj] = w[di + 1, dj + 1]
    return W


def _pack_windows(grids, agent_pos, agent_dir):
    """[N,25,25,3],[N,2],[N] -> [N,49] u8 rotated FPV window bytes."""
    N = grids.shape[0]
    ch0 = grids[..., 0]
    ch2 = grids[..., 2]
    closed = ((ch0 == 2) | (ch2 == 1)).astype(np.uint8)
    v = (ch0 | (grids[..., 1] << 2) | (ch2 << 4)).astype(np.uint8) \
        | (closed << 6)
    v = np.pad(v, ((0, 0), (5, 5), (5, 5)), constant_values=106)
    top_offset = np.array([[0, -3], [-3, 0], [-6, -3], [-3, -6]], np.int32)
    top = agent_pos + top_offset[agent_dir] + 5
    ii = top[:, 0, None, None] + np.arange(7, dtype=np.int32)[None, :, None]
    jj = top[:, 1, None, None] + np.arange(7, dtype=np.int32)[None, None, :]
    crop = v[np.arange(N)[:, None, None], ii, jj]
    rots = np.stack([np.rot90(crop, k, axes=(2, 1)) for k in range(4)])
    kmap = np.array([1, 2, 3, 0])
    crop = rots[kmap[agent_dir], np.arange(N)]
    return crop.reshape(N, 49)


def _install_ntff_hook():
    """Register the axon NTFF profile hook that boot() skips when
    antenv.axon_hooks is absent from the image. Trace-path only."""
    import sys
    import types
    if "antenv.axon_hooks" not in sys.modules:
        mod = types.ModuleType("antenv.axon_hooks")
        store = []
        mod.set_axon_ntff_profile_hook = store.append
        mod.get_axon_ntff_profile_hook = lambda: store[-1] if store else None
        import antenv
        sys.modules["antenv.axon_hooks"] = mod
        antenv.axon_hooks = mod
    mod = sys.modules["antenv.axon_hooks"]
    if mod.get_axon_ntff_profile_hook() is None:
        from trn_agent_boot.trn_boot import _ntff_profile_via_ctypes
        hook = _ntff_profile_via_ctypes("/opt/axon/libaxon_pjrt.so")
        if hook is not None:
            mod.set_axon_ntff_profile_hook(hook)
    # zero-egress container: keep artifacts local
    from concourse import bass_utils as _bu
    _bu.upload_artifacts = lambda d: d


_NC_CACHE = []


def kernel(grids, agent_pos, agent_dir, weight):
    grids = np.asarray(grids)
    agent_pos = np.ascontiguousarray(np.asarray(agent_pos, np.int32))
    agent_dir = np.ascontiguousarray(np.asarray(agent_dir, np.int32))
    N = grids.shape[0]
    ncores = 8
    per = N // ncores
    assert per == NENV, (N, NENV)

    W = _conv_matrix(weight)
    lhs_it = np.zeros((KR, 2 * KB), np.float32)
    lhs_it[0:KB, 0:KB] = -0.01 * W          # A: closed rows
    lhs_it[TB:KR, 0:KB] = W                 # A: t rows
    lhs_it[0:KB, KB:2 * KB] = W             # B: t rows
    lhs_it[TB:KR, KB:2 * KB] = -0.01 * W    # B: closed rows
    lhs_fin = np.zeros((KR, KB), np.float32)
    lhs_fin[0:KB] = W
    lhs_fin[TB:KR] = W
    lhs_fin = lhs_fin.astype(ml_dtypes.bfloat16)
    w27 = np.zeros((KR, 1), np.float32)
    w27[0:KB, 0] = np.tanh(W[:, 27])
    w27[TB:KR, 0] = np.tanh(W[:, 27])

    wins = _pack_windows(grids, agent_pos, agent_dir)  # [N, 49] u8

    in_maps = []
    for c in range(ncores):
        sl = slice(c * per, (c + 1) * per)
        in_maps.append({
            "cm": np.ascontiguousarray(wins[sl].T),
            "lhs_it": lhs_it,
            "lhs_fin": lhs_fin,
            "w27": w27,
        })

    nc = _NC_CACHE[0] if _NC_CACHE else build_nc()
    if not _NC_CACHE:
        _NC_CACHE.append(nc)

    trace = bool(int(os.environ.get("KERNEL_TRACE", "0")))
    if trace:
        try:
            _install_ntff_hook()
        except Exception as e:  # tracing is best-effort
            print(f"ntff hook install failed: {e}")
    r = run_bass_kernel_spmd(nc, in_maps, core_ids=list(range(ncores)),
                             trace=trace)
    LAST_RESULTS["bass"] = r
    outs = []
    for res in r.results:
        o = res["out"].reshape(KB, per).T  # [4096, 49] u8
        o = o.astype(np.int32)
        ch = np.stack([o & 3, (o >> 2) & 3, (o >> 4) & 3], axis=-1)
        outs.append(ch.reshape(per, 7, 7, 3).astype(np.int32))
    return np.concatenate(outs, axis=0)


# revision 5
# speedup vs baseline: 2.2301x; 2.2301x over previous
"""Trainium2 Bass kernel for nn_BatchMinigrid: batched FPV render.

v2 strategy (per core, 4096 envs), pure data parallel across 8 cores:

- Host packs each rotated 7x7 window cell into ONE byte
  (v = ch0 | ch1<<2 | ch2<<4 | closed<<6, wall pad byte = 106) and uploads
  it CELL-MAJOR: cm[cell 0:49, env 0:4096] u8. All value math (closed
  thresholds, the 5-step tanh visibility fixed point in f32, the final
  conv + masking and the masked output bytes) runs on device; the host
  contributes layout/addressing only (crop + rot90 byte moves), extending
  the v1 envelope (v1 already packed bytes + computed DMA indices on host).
  This removes the 32-call SWDGE indirect gather spine (~35.5us of Pool
  time at 994ns fixed cost per 128-descriptor call -- measured; multi-
  column offset APs return garbage on HW, and InstDMAGatherAnt cannot be
  encoded by walrus outside Bacc), and removes all 80 PE transposes.

- Dual-band cell-major layout, KR=98 rows: pair (A=even, B=odd unit):
    stA: rows 0:49  = closed_A (f32), rows 49:98 = t_A
    stB: rows 0:49  = t_B,            rows 49:98 = closed_B
  One PSUM z tile + one ACT tanh [0:98] serves BOTH units per step.
  lhs_it [98,98]: cols 0:49 = [-0.01W; W] (A), cols 49:98 = [W; -0.01W].

- SBUF cm tile holds TWO band copies; the second is written with a 512-col
  shift so ONE tensor_scalar over rows 0:98 at col X=512(2q+1) yields the
  pair's open mask (rows 0:49 = open_B at X, rows 49:98 = open_A at X-512).

- Final: per-band 49-row matmuls (bf16, sign-exact) into PSUM, then one
  fused scalar_tensor_tensor per unit: out_u8 = (zf > 0) * cm_byte.
  Output [49, 4096] u8 cell-major; host transposes + unpacks channels
  ((m*v)>>2k & 3 == m*ch_k exactly since the mask is 0/1).
"""
import os
import numpy as np
import ml_dtypes
from contextlib import ExitStack

import concourse.bass as bass
import concourse.tile as tile
from concourse import mybir
from concourse.bass_utils import run_bass_kernel_spmd

P = 128
NENV = 4096          # envs per core
EB = 512             # envs per matmul column block (unit)
NU = 8               # units per core
KB = 49              # band rows (window cells)
TB = 64              # upper band base (PE tile bases must be 0/32/64)
KR = 113             # contraction rows (49 + 15 zero + 49)

LAST_RESULTS = {}    # test harness introspection


# ----------------------------------------------------------------- waitsplit
def _split_excess_waits(nc, limit=1):
    n_split = 0
    for fn in nc.m.functions:
        for blk in fn.blocks:
            insts = blk.instructions
            i = 0
            while i < len(insts):
                inst = insts[i]
                si = getattr(inst, "sync_info", None)
                if si is not None and si.on_wait and len(si.on_wait) > limit:
                    waits = list(si.on_wait)
                    si.on_wait.clear()
                    si.on_wait.extend(waits[-limit:])
                    rest = waits[:-limit]
                    pos = i
                    for j in range(0, len(rest), limit):
                        nop = mybir.InstNoOp(
                            name=f"{inst.name}_wsplit{j}",
                            engine=inst.engine,
                            bass_nofuse=True,
                            sync_info=mybir.SyncInfo(
                                on_wait=rest[j:j + limit], on_update=[]),
                        )
                        insts.insert(pos, nop)
                        pos += 1
                        i += 1
                        n_split += 1
                i += 1
    return n_split


# ----------------------------------------------------------------- builder
def build_nc():
    f32 = mybir.dt.float32
    bf16 = mybir.dt.bfloat16
    u8 = mybir.dt.uint8
    nc = bass.Bass()

    cm = nc.dram_tensor("cm", [KB, NENV], u8, kind="ExternalInput")
    lhs_it = nc.dram_tensor("lhs_it", [KR, 2 * KB], mybir.dt.float32,
                            kind="ExternalInput")
    lhs_fin = nc.dram_tensor("lhs_fin", [KR, KB], bf16, kind="ExternalInput")
    w27 = nc.dram_tensor("w27", [KR, 1], mybir.dt.float32,
                         kind="ExternalInput")
    out = nc.dram_tensor("out", [KB, NENV], u8, kind="ExternalOutput")

    with tile.TileContext(nc) as tc, ExitStack() as ctx:
        const = ctx.enter_context(tc.tile_pool(name="const", bufs=1))
        cmp_ = ctx.enter_context(tc.tile_pool(name="cmp", bufs=1))
        stp = ctx.enter_context(tc.tile_pool(name="stp", bufs=1))
        opsp = ctx.enter_context(tc.tile_pool(name="opsp", bufs=1))
        thp = ctx.enter_context(tc.tile_pool(name="thp", bufs=3))
        outp = ctx.enter_context(tc.tile_pool(name="outp", bufs=1))
        psZ = ctx.enter_context(tc.tile_pool(name="psZ", bufs=4, space="PSUM"))
        psF = ctx.enter_context(tc.tile_pool(name="psF", bufs=4, space="PSUM"))

        TS = nc.vector.tensor_scalar
        GTS = nc.gpsimd.tensor_scalar
        TT = nc.vector.tensor_tensor
        GTT = nc.gpsimd.tensor_tensor
        STT = nc.vector.scalar_tensor_tensor
        Alu = mybir.AluOpType
        ACTF = mybir.ActivationFunctionType

        # ---------------- inputs
        lhs_it_t = const.tile([P, 2 * KB], f32)
        nc.sync.dma_start(out=lhs_it_t[0:KR, :], in_=lhs_it[:])
        lhs_fin_t = const.tile([P, KB], bf16)
        nc.sync.dma_start(out=lhs_fin_t[0:KR, :], in_=lhs_fin[:])
        w27_t = const.tile([P, 1], f32)
        nc.sync.dma_start(out=w27_t[0:KR, :], in_=w27[:])

        # cm tile: band1 rows 0:49 = cm[:, c]; band2 rows 49:98 shifted by
        # +512 cols (cmt[49:98, c] = cm[:, c-512]) so pair-level 98-row APs
        # line up (see module docstring).
        cmt = cmp_.tile([P, NENV + EB], u8)
        nc.sync.dma_start(out=cmt[0:KB, 0:NENV], in_=cm[:])
        nc.sync.dma_start(out=cmt[TB:KR, EB:NENV + EB], in_=cm[:])

        sts = [None] * NU
        opss = [None] * 4
        mbs = [None] * NU
        outbuf = outp.tile([P, NENV], u8)

        # ---------------- front end per pair (quad q = units 2q, 2q+1)
        def front(q):
            ua, ub = 2 * q, 2 * q + 1
            xa, xb = ua * EB, ub * EB
            stA = stp.tile([P, EB], f32, tag=f"st{ua}", name=f"st{ua}")
            stB = stp.tile([P, EB], f32, tag=f"st{ub}", name=f"st{ub}")
            ops = opsp.tile([P, EB], bf16, tag=f"ops{q}", name=f"ops{q}")
            sts[ua], sts[ub] = stA, stB
            opss[q] = ops
            with nc.named_scope("front"):
                # zero gap band first (rows 32:49 rewritten by closed below;
                # engine partition bases must be quarter-aligned)
                nc.scalar.memzero(stA[32:TB, :])
                nc.scalar.memzero(stB[32:TB, :])
                # closed bands straight to f32 st
                TS(out=stA[0:KB, :], in0=cmt[0:KB, xa:xa + EB],
                   scalar1=64.0, scalar2=None, op0=Alu.is_ge)
                TS(out=stB[TB:KR, :], in0=cmt[TB:KR, xb + EB:xb + 2 * EB],
                   scalar1=64.0, scalar2=None, op0=Alu.is_ge)
                # pair open mask in one op (band trick)
                TS(out=ops[0:KR, :], in0=cmt[0:KR, xb:xb + EB],
                   scalar1=64.0, scalar2=None, op0=Alu.is_lt)
                # t-band init: tanh(W[:,27]) * open
                nc.scalar.activation(
                    out=stA[TB:KR, :], in_=ops[TB:KR, :],
                    func=ACTF.Copy, scale=w27_t[TB:KR, :])
                nc.scalar.activation(
                    out=stB[0:KB, :], in_=ops[0:KB, :],
                    func=ACTF.Copy, scale=w27_t[0:KB, :])

        # ---------------- iterations
        def iter_l(l, q):
            ua, ub = 2 * q, 2 * q + 1
            fn = ACTF.Tanh if l < 5 else ACTF.Relu
            with nc.named_scope(f"iter{l}"):
                zq = psZ.tile([P, EB], mybir.dt.float32, tag="z",
                              name=f"z{q}_{l}")
                th = thp.tile([P, EB], mybir.dt.float32, tag="th",
                              name=f"th{q}_{l}")
                if l < 5:
                    outa, outb = sts[ua], sts[ub]
                else:
                    outa = stp.tile([P, EB], bf16, tag=f"mb{ua}",
                                    name=f"mb{ua}")
                    outb = stp.tile([P, EB], bf16, tag=f"mb{ub}",
                                    name=f"mb{ub}")
                    mbs[ua], mbs[ub] = outa, outb
                nc.tensor.matmul(
                    out=zq[TB:KR, :], lhsT=lhs_it_t[0:KR, 0:KB],
                    rhs=sts[ua][0:KR, :], start=True, stop=True)
                nc.tensor.matmul(
                    out=zq[0:KB, :], lhsT=lhs_it_t[0:KR, KB:2 * KB],
                    rhs=sts[ub][0:KR, :], start=True, stop=True)
                nc.scalar.activation(out=th[0:KR, :], in_=zq[0:KR, :],
                                     func=fn)
                TT(out=outa[TB:KR, :], in0=th[TB:KR, :],
                   in1=opss[q][TB:KR, :], op=Alu.mult)
                TT(out=outb[0:KB, :], in0=th[0:KB, :],
                   in1=opss[q][0:KB, :], op=Alu.mult)

        # ---------------- final conv + fused mask-multiply
        def final(q):
            ua, ub = 2 * q, 2 * q + 1
            xa, xb = ua * EB, ub * EB
            with nc.named_scope("final"):
                zfA = psF.tile([P, EB], mybir.dt.float32, tag="zf",
                               name=f"zfA{q}")
                zfB = psF.tile([P, EB], mybir.dt.float32, tag="zf",
                               name=f"zfB{q}")
                nc.tensor.matmul(
                    out=zfA[0:KB, :], lhsT=lhs_fin_t[TB:KR, :],
                    rhs=mbs[ua][TB:KR, :], start=True, stop=True)
                nc.tensor.matmul(
                    out=zfB[0:KB, :], lhsT=lhs_fin_t[0:KB, :],
                    rhs=mbs[ub][0:KB, :], start=True, stop=True)
                STT(out=outbuf[0:KB, xa:xa + EB], in0=zfA[0:KB, :],
                    scalar=0.0, in1=cmt[0:KB, xa:xa + EB],
                    op0=Alu.is_gt, op1=Alu.mult)
                STT(out=outbuf[0:KB, xb:xb + EB], in0=zfB[0:KB, :],
                    scalar=0.0, in1=cmt[0:KB, xb:xb + EB],
                    op0=Alu.is_gt, op1=Alu.mult)
                nc.sync.dma_start(out=out[:, xa:xa + 2 * EB],
                                  in_=outbuf[0:KB, xa:xa + 2 * EB])

        for q in range(4):
            front(q)
        # wavefront: quad q's layer l at wave q + l, finals at wave q + 6
        for w in range(2, 10):
            for q in range(4):
                l = w - q
                if 2 <= l <= 5:
                    iter_l(l, q)
                elif l == 6:
                    final(q)

    _split_excess_waits(nc)
    return nc


# ----------------------------------------------------------------- host side
def _conv_matrix(w):
    w = np.asarray(w, np.float32).reshape(3, 3)
    W = np.zeros((49, 49), np.float32)
    for i in range(7):
        for j in range(7):
            for di in (-1, 0, 1):
                for dj in (-1, 0, 1):
                    ii, jj = i + di, j + dj
                    if 0 <= ii < 7 and 0 <= jj < 7:
                        W[i * 7 + j, ii * 7 + jj] = w[di + 1, dj + 1]
    return W


def _pack_windows(grids, agent_pos, agent_dir):
    """[N,25,25,3],[N,2],[N] -> [N,49] u8 rotated FPV window bytes."""
    N = grids.shape[0]
    ch0 = grids[..., 0]
    ch2 = grids[..., 2]
    closed = ((ch0 == 2) | (ch2 == 1)).astype(np.uint8)
    v = (ch0 | (grids[..., 1] << 2) | (ch2 << 4)).astype(np.uint8) \
        | (closed << 6)
    v = np.pad(v, ((0, 0), (5, 5), (5, 5)), constant_values=106)
    top_offset = np.array([[0, -3], [-3, 0], [-6, -3], [-3, -6]], np.int32)
    top = agent_pos + top_offset[agent_dir] + 5
    ii = top[:, 0, None, None] + np.arange(7, dtype=np.int32)[None, :, None]
    jj = top[:, 1, None, None] + np.arange(7, dtype=np.int32)[None, None, :]
    crop = v[np.arange(N)[:, None, None], ii, jj]
    rots = np.stack([np.rot90(crop, k, axes=(2, 1)) for k in range(4)])
    kmap = np.array([1, 2, 3, 0])
    crop = rots[kmap[agent_dir], np.arange(N)]
    return crop.reshape(N, 49)


def _install_ntff_hook():
    """Register the axon NTFF profile hook that boot() skips when
    antenv.axon_hooks is absent from the image. Trace-path only."""
    import sys
    import types
    if "antenv.axon_hooks" not in sys.modules:
        mod = types.ModuleType("antenv.axon_hooks")
        store = []
        mod.set_axon_ntff_profile_hook = store.append
        mod.get_axon_ntff_profile_hook = lambda: store[-1] if store else None
        import antenv
        sys.modules["antenv.axon_hooks"] = mod
        antenv.axon_hooks = mod
    mod = sys.modules["antenv.axon_hooks"]
    if mod.get_axon_ntff_profile_hook() is None:
        from trn_agent_boot.trn_boot import _ntff_profile_via_ctypes
        hook = _ntff_profile_via_ctypes("/opt/axon/libaxon_pjrt.so")
        if hook is not None:
            mod.set_axon_ntff_profile_hook(hook)
    # zero-egress container: keep artifacts local
    from concourse import bass_utils as _bu
    _bu.upload_artifacts = lambda d: d


_NC_CACHE = []


def kernel(grids, agent_pos, agent_dir, weight):
    grids = np.asarray(grids)
    agent_pos = np.ascontiguousarray(np.asarray(agent_pos, np.int32))
    agent_dir = np.ascontiguousarray(np.asarray(agent_dir, np.int32))
    N = grids.shape[0]
    ncores = 8
    per = N // ncores
    assert per == NENV, (N, NENV)

    W = _conv_matrix(weight)
    lhs_it = np.zeros((KR, 2 * KB), np.float32)
    lhs_it[0:KB, 0:KB] = -0.01 * W          # A: closed rows
    lhs_it[TB:KR, 0:KB] = W                 # A: t rows
    lhs_it[0:KB, KB:2 * KB] = W             # B: t rows
    lhs_it[TB:KR, KB:2 * KB] = -0.01 * W    # B: closed rows
    lhs_fin = np.zeros((KR, KB), np.float32)
    lhs_fin[0:KB] = W
    lhs_fin[TB:KR] = W
    lhs_fin = lhs_fin.astype(ml_dtypes.bfloat16)
    w27 = np.zeros((KR, 1), np.float32)
    w27[0:KB, 0] = np.tanh(W[:, 27])
    w27[TB:KR, 0] = np.tanh(W[:, 27])

    wins = _pack_windows(grids, agent_pos, agent_dir)  # [N, 49] u8

    in_maps = []
    for c in range(ncores):
        sl = slice(c * per, (c + 1) * per)
        in_maps.append({
            "cm": np.ascontiguousarray(wins[sl].T),
            "lhs_it": lhs_it,
            "lhs_fin": lhs_fin,
            "w27": w27,
        })

    nc = _NC_CACHE[0] if _NC_CACHE else build_nc()
    if not _NC_CACHE:
        _NC_CACHE.append(nc)

    trace = bool(int(os.environ.get("KERNEL_TRACE", "0")))
    if trace:
        try:
            _install_ntff_hook()
        except Exception as e:  # tracing is best-effort
            print(f"ntff hook install failed: {e}")
    r = run_bass_kernel_spmd(nc, in_maps, core_ids=list(range(ncores)),
                             trace=trace)
    LAST_RESULTS["bass"] = r
    outs = []
    for res in r.results:
        o = res["out"].reshape(KB, per).T  # [4096, 49] u8
        o = o.astype(np.int32)
        ch = np.stack([o & 3, (o >> 2) & 3, (o >> 4) & 3], axis=-1)
        outs.append(ch.reshape(per, 7, 7, 3).astype(np.int32))
    return np.concatenate(outs, axis=0)
